# revision 1
# baseline (speedup 1.0000x reference)
"""Trainium2 Bass kernel for 3-layer GATv2 + sum-pool + MLP (nn_GAT_56977036148745).

Strategy (8 NeuronCores, SPMD):
  - Nodes sharded into 8 contiguous slabs of 2048 (dst-sharding). Each core owns
    all edges whose destination lands in its slab (edges sorted by dst on host).
  - Per layer: every core builds its slab of the gather table T = h @ Wl (+bias
    row), an AllGather assembles the full [N, 256] bf16 table in each core's
    DRAM. Edge phase gathers source rows twice per 128-dst tile (channel-major
    via transposing dma_gather for the logit path, edge-major for aggregation).
  - Logits: m = xl[src] + xr[dst] (PE one-hot broadcast + identity-add into
    PSUM), t = LeakyRelu(m) on ACT, per-edge logit = att . t via tiny PE
    matmuls.  exp on ACT (no max subtraction needed; logits are O(5)).
  - Aggregation: per 128-edge chunk, M[e,d] = (dst_e == d) * exp(logit_e) built
    by one DVE tensor_scalar (iota/is_equal/mult), then PSUM-accumulated
    seg-matmuls give numerator and denominator per dst tile.
  - Sum-pool accumulated per layer with graph-one-hot matmuls; partial pools
    scattered into a [G+1, 772] bounce and AllReduced (graphs may straddle core
    boundaries). MLP runs redundantly on every core.
Host preprocessing (sorting, padding, index wrapping, weight folding) is not
part of the measured device time.
"""

import sys

for _p in ("/opt/trn_rl_repo", "/root/.axon_site/_ro/trn_rl_repo"):
    if _p not in sys.path:
        sys.path.append(_p)

import numpy as np
import ml_dtypes

try:  # NTFF profiling hook shim (image's antenv lacks axon_hooks)
    import antenv.axon_hooks  # noqa: F401
except ImportError:
    import types as _types

    try:
        import trn_agent_boot.trn_boot as _tb
        _ntff_hook = _tb._ntff_profile_via_ctypes("/opt/axon/libaxon_pjrt.so")
    except Exception:
        _ntff_hook = None
    _m = _types.ModuleType("antenv.axon_hooks")
    _m.get_axon_ntff_profile_hook = lambda: _ntff_hook
    _m.set_axon_ntff_profile_hook = lambda h: None
    sys.modules["antenv.axon_hooks"] = _m

import concourse.bacc as bacc
import concourse.bass as bass
import concourse.mybir as mybir
import concourse.tile as tile
from concourse.bass import IndirectOffsetOnAxis
from concourse.bass_utils import run_bass_kernel_spmd

BF = ml_dtypes.bfloat16
F32 = mybir.dt.float32
BF16 = mybir.dt.bfloat16
I16 = mybir.dt.int16
I32 = mybir.dt.int32

P = 128          # partitions / dst-tile size / edge-chunk size
H = 2            # heads
C = 128          # channels per head
D = H * C        # 256
AF = mybir.ActivationFunctionType
ALU = mybir.AluOpType
NEG_SLOPE = 0.2


class Cfg:
    def __init__(self, n, g, ndev, nchunk, in_ch=128):
        self.n = n                  # total nodes
        self.g = g                  # graphs
        self.ndev = ndev
        self.nchunk = nchunk        # edge chunks (of 128) per dst tile, mult of 4
        self.in_ch = in_ch          # layer-1 input channels
        self.npd = n // ndev        # nodes per device
        self.nt = self.npd // P     # dst/node tiles per device
        self.pda = 772              # padded pool dim (768 + cnt + pad)


# ----------------------------------------------------------------------------
# device program
# ----------------------------------------------------------------------------

def build_program(cfg: Cfg, fb2: float):
    nc = bacc.Bacc("TRN2", target_bir_lowering=False, debug=False,
                   num_devices=cfg.ndev)
    NT, NC, G, NPD = cfg.nt, cfg.nchunk, cfg.g, cfg.npd
    NE = NC * P                      # padded edges per dst tile
    KC1 = cfg.in_ch // P             # layer-1 K chunks (1)
    PDA = cfg.pda

    dt = nc.dram_tensor
    xT_d = dt("xT", [cfg.in_ch, NPD], BF16, kind="ExternalInput")
    wl_d = [dt(f"wl{l}", [cfg.in_ch if l == 0 else D, D], BF16, kind="ExternalInput") for l in range(3)]
    wr_d = [dt(f"wr{l}", [cfg.in_ch if l == 0 else D, D], BF16, kind="ExternalInput") for l in range(3)]
    rl_d = [None] + [dt(f"rl{l}", [1, D], BF16, kind="ExternalInput") for l in (1, 2)]
    rr_d = [None] + [dt(f"rr{l}", [1, D], BF16, kind="ExternalInput") for l in (1, 2)]
    att_d = dt("att", [C, 2 * 3], BF16, kind="ExternalInput")
    idxw_d = dt("idxw", [NT, P, NC // 4, 32], I16, kind="ExternalInput")
    dstloc_d = dt("dstloc", [NT, P, NC], F32, kind="ExternalInput")
    dstrow_d = dt("dstrow", [NT, NE], BF16, kind="ExternalInput")
    bloc_d = dt("bloc", [P, NT], F32, kind="ExternalInput")
    poolidx_d = dt("poolidx", [P, 1], I32, kind="ExternalInput")
    fW1_d = dt("fW1p", [PDA, 768], F32, kind="ExternalInput")
    fb1_d = dt("fb1p", [P, 6], F32, kind="ExternalInput")
    fW2_d = dt("fW2p", [P, 6], F32, kind="ExternalInput")
    identbf_d = dt("identbf", [P, P], BF16, kind="ExternalInput")
    identf_d = dt("identf", [P, P], F32, kind="ExternalInput")
    iotarow_d = dt("iotarow", [P, P], BF16, kind="ExternalInput")
    iotacol_d = dt("iotacol", [P, 1], F32, kind="ExternalInput")
    ones1p_d = dt("ones1p", [1, P], BF16, kind="ExternalInput")
    onescol_d = dt("onescol", [P, 1], BF16, kind="ExternalInput")
    y_d = dt("y", [1, G], F32, kind="ExternalOutput")

    rg = [list(range(cfg.ndev))]

    with tile.TileContext(nc) as tc:
        with (
            tc.tile_pool(name="persist", bufs=1) as pp,
            tc.tile_pool(name="dram", bufs=2, space="DRAM") as dram,
            tc.tile_pool(name="gath", bufs=2) as gpool,
            tc.tile_pool(name="work", bufs=2) as wpool,
            tc.tile_pool(name="mtile", bufs=3) as mpool,
            tc.tile_pool(name="pm", bufs=1, space="PSUM") as pmpool,
            tc.tile_pool(name="pseg", bufs=2, space="PSUM") as psegpool,
            tc.tile_pool(name="pdst", bufs=1, space="PSUM") as pdstpool,
            tc.tile_pool(name="plog", bufs=1, space="PSUM") as plogpool,
            tc.tile_pool(name="pmisc", bufs=1, space="PSUM") as pmiscpool,
        ):
            # ---- persistent SBUF ----
            identbf = pp.tile([P, P], BF16, tag="identbf")
            identf = pp.tile([P, P], F32, tag="identf")
            iotarow = pp.tile([P, P], BF16, tag="iotarow")
            iotacol = pp.tile([P, 1], F32, tag="iotacol")
            ones1p = pp.tile([1, P], BF16, tag="ones1p")
            onescol = pp.tile([P, 1], BF16, tag="onescol")
            att_sb = pp.tile([C, 6], BF16, tag="att")
            xT_sb = pp.tile([cfg.in_ch, NPD], BF16, tag="xT")
            hT_sb = pp.tile([P, 2, NPD], BF16, tag="hT")
            xr_sb = pp.tile([P, NT, D], BF16, tag="xr")
            pool_sb = pp.tile([P, PDA], F32, tag="pool")
            bloc_sb = pp.tile([P, NT], F32, tag="bloc")
            poolidx_sb = pp.tile([P, 1], I32, tag="poolidx")
            wl_sb = [pp.tile([P, (cfg.in_ch if l == 0 else D) // P, D], BF16, name=f"wl{l}", tag=f"wl{l}") for l in range(3)]
            wr_sb = [pp.tile([P, (cfg.in_ch if l == 0 else D) // P, D], BF16, name=f"wr{l}", tag=f"wr{l}") for l in range(3)]
            rl_sb = [None, pp.tile([1, D], BF16, name="rl1", tag="rl1"), pp.tile([1, D], BF16, name="rl2", tag="rl2")]
            rr_sb = [None, pp.tile([1, D], BF16, name="rr1", tag="rr1"), pp.tile([1, D], BF16, name="rr2", tag="rr2")]
            zero_sb = pp.tile([P, PDA], F32, tag="zero")

            for sb, d in ((identbf, identbf_d), (identf, identf_d),
                          (iotarow, iotarow_d), (iotacol, iotacol_d),
                          (ones1p, ones1p_d), (onescol, onescol_d),
                          (att_sb, att_d), (xT_sb, xT_d), (bloc_sb, bloc_d),
                          (poolidx_sb, poolidx_d)):
                nc.sync.dma_start(sb[:], d[:])
            for l in range(3):
                kc = (cfg.in_ch if l == 0 else D) // P
                nc.sync.dma_start(wl_sb[l][:], wl_d[l].ap().rearrange("(k p) d -> p k d", p=P))
                nc.sync.dma_start(wr_sb[l][:], wr_d[l].ap().rearrange("(k p) d -> p k d", p=P))
                if l > 0:
                    nc.sync.dma_start(rl_sb[l][:], rl_d[l][:])
                    nc.sync.dma_start(rr_sb[l][:], rr_d[l][:])
            nc.vector.memset(pool_sb[:], 0.0)
            nc.vector.memset(zero_sb[:], 0.0)

            # pool bounce (zeroed before scatter)
            poolb_in = dram.tile([G + 1, PDA], F32, tag="poolb_in")
            poolb_out = dram.tile([G + 1, PDA], F32, tag="poolb_out")
            for r0 in range(0, G + 1, P):
                rows = min(P, G + 1 - r0)
                nc.sync.dma_start(poolb_in[r0:r0 + rows, :], zero_sb[:rows, :])

            # ------------------------------------------------------------------
            for l in range(3):
                kcs = KC1 if l == 0 else 2
                src_sb = xT_sb if l == 0 else hT_sb

                def src_lhsT(kc, nt):
                    if l == 0:
                        return src_sb[:, nt * P:(nt + 1) * P]
                    return src_sb[:, kc, nt * P:(nt + 1) * P]

                # ---- table slab + XR slab ----
                slab = dram.tile([NPD, D], BF16, tag="slab")
                Tfull = dram.tile([cfg.n, D], BF16, tag="Tfull")
                for nt in range(NT):
                    ptab = pmiscpool.tile([P, D], F32, tag="ptab")
                    for kc in range(kcs):
                        nc.tensor.matmul(ptab[:], src_lhsT(kc, nt), wl_sb[l][:, kc, :],
                                         start=(kc == 0), stop=(kc == kcs - 1 and l == 0))
                    if l > 0:
                        nc.tensor.matmul(ptab[:], ones1p[:], rl_sb[l][:], start=False, stop=True)
                    tab = wpool.tile([P, D], BF16, tag="tab")
                    nc.vector.tensor_copy(tab[:], ptab[:])
                    nc.sync.dma_start(slab[nt * P:(nt + 1) * P, :], tab[:])

                    pxr = pmiscpool.tile([P, D], F32, tag="ptab")
                    for kc in range(kcs):
                        nc.tensor.matmul(pxr[:], src_lhsT(kc, nt), wr_sb[l][:, kc, :],
                                         start=(kc == 0), stop=(kc == kcs - 1 and l == 0))
                    if l > 0:
                        nc.tensor.matmul(pxr[:], ones1p[:], rr_sb[l][:], start=False, stop=True)
                    nc.vector.tensor_copy(xr_sb[:, nt, :], pxr[:])

                nc.gpsimd.collective_compute(
                    "AllGather", ALU.bypass, replica_groups=rg,
                    ins=[slab.opt()], outs=[Tfull.opt()],
                )

                # ---- edge phase ----
                ppool = pmiscpool.tile([P, D + 1], F32, tag="ppool")
                for t in range(NT):
                    idx_sb = wpool.tile([P, NC // 4, 32], I16, tag="idx")
                    nc.sync.dma_start(idx_sb[:], idxw_d[t])
                    dstrow = wpool.tile([1, NE], BF16, tag="dstrow")
                    nc.sync.dma_start(dstrow[:], dstrow_d[t:t + 1, :])
                    dstloc = wpool.tile([P, NC], F32, tag="dstloc")
                    nc.sync.dma_start(dstloc[:], dstloc_d[t])

                    xlE = gpool.tile([P, NC, D], BF16, tag="xlE")
                    for g in range(NC // 4):
                        nc.gpsimd.dma_gather(xlE[:, 4 * g:4 * (g + 1), :], Tfull[:],
                                             idx_sb[:, g, :], 512, 512, D)

                    pseg = psegpool.tile([P, 2 * (C + 1)], F32, tag="pseg")
                    plog = plogpool.tile([P, 2 * NC], F32, tag="plog")

                    for g in range(NC // 4):
                        e0 = g * 512
                        xlT = gpool.tile([P, H, 512], BF16, tag="xlT")
                        nc.gpsimd.dma_gather(xlT[:], Tfull[:], idx_sb[:, g, :],
                                             512, 512, D, transpose=True)
                        pdst = pdstpool.tile([P, 512], F32, tag="pdst")
                        nc.tensor.matmul(pdst[:], ones1p[:], dstrow[:, e0:e0 + 512],
                                         start=True, stop=True)
                        onehot = wpool.tile([P, 512], BF16, tag="onehot")
                        nc.vector.tensor_scalar(onehot[:], pdst[:], iotacol[:], None,
                                                ALU.is_equal)
                        pm = pmpool.tile([P, H, 512], F32, tag="pm")
                        for h in range(H):
                            nc.tensor.matmul(pm[:, h, :],
                                             xr_sb[:, t, h * C:(h + 1) * C],
                                             onehot[:], start=True, stop=False)
                            nc.tensor.matmul(pm[:, h, :], identbf[:],
                                             xlT[:, h, :],
                                             start=False, stop=True)
                        tsb = wpool.tile([P, H, 512], BF16, tag="tsb")
                        nc.scalar.activation(tsb[:], pm[:], AF.Prelu, alpha=NEG_SLOPE)
                        for h in range(H):
                            for sub in range(4):
                                k = g * 4 + sub
                                nc.tensor.matmul(
                                    plog[:, 2 * k + h:2 * k + h + 1],
                                    tsb[:, h, sub * P:(sub + 1) * P],
                                    att_sb[:, l * 2 + h:l * 2 + h + 1],
                                    start=True, stop=True)

                    ev = wpool.tile([P, 2 * NC], F32, tag="ev")
                    nc.scalar.activation(ev[:], plog[:], AF.Exp)

                    for k in range(NC):
                        for h in range(H):
                            Mt = mpool.tile([P, P], BF16, tag="Mt")
                            nc.vector.tensor_scalar(
                                Mt[:], iotarow[:], dstloc[:, k:k + 1],
                                ev[:, 2 * k + h:2 * k + h + 1],
                                ALU.is_equal, ALU.mult)
                            base = h * (C + 1)
                            nc.tensor.matmul(pseg[:, base:base + C], Mt[:],
                                             xlE[:, k, h * C:(h + 1) * C],
                                             start=(k == 0 and h == 0), stop=False)
                            nc.tensor.matmul(pseg[:, base + C:base + C + 1], Mt[:],
                                             onescol[:], start=False,
                                             stop=(k == NC - 1 and h == 1))

                    rec = wpool.tile([P, 2], F32, tag="rec")
                    hst = wpool.tile([P, D], BF16, tag="hst")
                    for h in range(H):
                        base = h * (C + 1)
                        nc.vector.reciprocal(rec[:, h:h + 1], pseg[:, base + C:base + C + 1])
                        nc.vector.tensor_scalar(hst[:, h * C:(h + 1) * C],
                                                pseg[:, base:base + C],
                                                rec[:, h:h + 1], None, ALU.mult)

                    # pooling
                    Gt = wpool.tile([P, P], BF16, tag="Gt")
                    nc.vector.tensor_scalar(Gt[:], iotarow[:], bloc_sb[:, t:t + 1],
                                            None, ALU.is_equal)
                    nc.tensor.matmul(ppool[:, :D], Gt[:], hst[:],
                                     start=(t == 0), stop=(t == NT - 1 and l != 0))
                    if l == 0:
                        nc.tensor.matmul(ppool[:, D:D + 1], Gt[:], onescol[:],
                                         start=False, stop=(t == NT - 1))

                    # transpose h for next layer's table build
                    if l < 2:
                        for h in range(H):
                            ptr = pmiscpool.tile([P, P], BF16, tag="ptab")
                            nc.tensor.transpose(ptr[:], hst[:, h * C:(h + 1) * C],
                                                identbf[:])
                            nc.vector.tensor_copy(hT_sb[:, h, t * P:(t + 1) * P], ptr[:])

                nc.vector.tensor_copy(pool_sb[:, l * D:(l + 1) * D], ppool[:, :D])
                if l == 0:
                    nc.vector.tensor_copy(pool_sb[:, 768:769], ppool[:, D:D + 1])

            # ------------------------------------------------------------------
            # pooling allreduce + MLP
            nc.gpsimd.indirect_dma_start(
                out=poolb_in[:],
                out_offset=IndirectOffsetOnAxis(ap=poolidx_sb[:, :1], axis=0),
                in_=pool_sb[:],
                in_offset=None,
            )
            nc.gpsimd.collective_compute(
                "AllReduce", ALU.add, replica_groups=rg,
                ins=[poolb_in.opt()], outs=[poolb_out.opt()],
            )

            fW1_sb = [pp.tile([P, 768], F32, name=f"fW1_{kc}", tag=f"fW1_{kc}") for kc in range(7)]
            for kc in range(7):
                kr = min(P, PDA - kc * P)
                nc.sync.dma_start(fW1_sb[kc][:kr, :], fW1_d[kc * P:kc * P + kr, :])
            fb1_sb = pp.tile([P, 6], F32, tag="fb1")
            nc.sync.dma_start(fb1_sb[:], fb1_d[:])
            fW2_sb = pp.tile([P, 6], F32, tag="fW2")
            nc.sync.dma_start(fW2_sb[:], fW2_d[:])

            poolT = [pp.tile([P, max(G, P)], F32, name=f"poolT_{kc}", tag=f"poolT_{kc}") for kc in range(7)]
            for rt in range(0, G, P):
                rows = min(P, G - rt)
                prow = wpool.tile([P, PDA], F32, tag="prow")
                nc.sync.dma_start(prow[:rows, :], poolb_out[rt:rt + rows, :])
                for cb in range(7):
                    w = min(P, PDA - cb * P)
                    ptr2 = pmiscpool.tile([P, P], F32, tag="ptab")
                    nc.tensor.transpose(ptr2[:w, :rows], prow[:rows, cb * P:cb * P + w],
                                        identf[:rows, :rows])
                    nc.vector.tensor_copy(poolT[cb][:w, rt:rt + rows], ptr2[:w, :rows])

            h1_sb = [pp.tile([P, max(G, P)], F32, name=f"h1_{mo}", tag=f"h1_{mo}") for mo in range(6)]
            for mo in range(6):
                ph1 = pmiscpool.tile([P, max(G, P)], F32, tag="ptab")
                for kc in range(7):
                    kr = min(P, PDA - kc * P)
                    nc.tensor.matmul(ph1[:, :G], fW1_sb[kc][:kr, mo * P:(mo + 1) * P],
                                     poolT[kc][:kr, :G], start=(kc == 0), stop=(kc == 6))
                nc.scalar.activation(h1_sb[mo][:, :G], ph1[:, :G], AF.Relu,
                                     bias=fb1_sb[:, mo:mo + 1])

            py = pmiscpool.tile([1, max(G, P)], F32, tag="ppool")
            for mo in range(6):
                nc.tensor.matmul(py[:, :G], fW2_sb[:, mo:mo + 1], h1_sb[mo][:, :G],
                                 start=(mo == 0), stop=(mo == 5))
            ysb = wpool.tile([1, max(G, P)], F32, tag="ysb")
            nc.vector.tensor_copy(ysb[:, :G], py[:, :G])
            nc.sync.dma_start(y_d[:], ysb[:1, :G])

    nc.compile()
    return nc


# ----------------------------------------------------------------------------
# host preprocessing
# ----------------------------------------------------------------------------

def preprocess(inputs: dict, cfg: Cfg):
    n, g, ndev = cfg.n, cfg.g, cfg.ndev
    NPD, NT = cfg.npd, cfg.nt

    x = np.asarray(inputs["x"], np.float32)
    ei = np.asarray(inputs["edge_index"]).astype(np.int64)
    batch = np.asarray(inputs["batch"]).astype(np.int64)

    src = np.concatenate([ei[0], np.arange(n)])
    dst = np.concatenate([ei[1], np.arange(n)])
    order = np.argsort(dst, kind="stable")
    src, dst = src[order], dst[order]

    # per (dev, tile) edge lists
    tile_of = dst // P              # global dst tile id (NT per device)
    counts = np.bincount(tile_of, minlength=(n // P))
    nchunk = int(np.ceil(counts.max() / P))
    nchunk = ((nchunk + 3) // 4) * 4
    cfg.nchunk = nchunk
    NE = nchunk * P

    tile_start = np.zeros(n // P + 1, np.int64)
    np.cumsum(counts, out=tile_start[1:])

    def wrap_idx(a):  # [512] int16 -> [128, 32]
        w = a.reshape(-1, 16).T.copy()          # [16, 32]
        return np.tile(w, (8, 1))               # [128, 32]

    in_maps = []
    consts = {
        "identbf": np.eye(P, dtype=BF),
        "identf": np.eye(P, dtype=np.float32),
        "iotarow": np.tile(np.arange(P, dtype=BF)[None, :], (P, 1)),
        "iotacol": np.arange(P, dtype=np.float32)[:, None],
        "ones1p": np.ones((1, P), BF),
        "onescol": np.ones((P, 1), BF),
    }
    att_all = np.stack([np.asarray(inputs[f"att{l+1}"], np.float32) for l in range(3)])  # [3, H, C]
    att_cols = np.zeros((C, 6), np.float32)
    for l in range(3):
        for h in range(H):
            att_cols[:, l * 2 + h] = att_all[l, h]

    b = [np.asarray(inputs[f"b{l+1}"], np.float32) for l in range(3)]
    wmats = {}
    for l in range(3):
        wmats[f"wl{l}"] = np.asarray(inputs[f"Wl{l+1}"], np.float32).astype(BF)
        wmats[f"wr{l}"] = np.asarray(inputs[f"Wr{l+1}"], np.float32).astype(BF)
    for l in (1, 2):
        wmats[f"rl{l}"] = (b[l - 1] @ np.asarray(inputs[f"Wl{l+1}"], np.float32))[None, :].astype(BF)
        wmats[f"rr{l}"] = (b[l - 1] @ np.asarray(inputs[f"Wr{l+1}"], np.float32))[None, :].astype(BF)

    b_all = np.concatenate(b)
    fW1 = np.asarray(inputs["fW1"], np.float32)
    fW1p = np.zeros((cfg.pda, 768), np.float32)
    fW1p[:768] = fW1
    fW1p[768] = b_all @ fW1
    fb1 = np.asarray(inputs["fb1"], np.float32).reshape(6, P).T.copy()
    fW2p = np.asarray(inputs["fW2"], np.float32).reshape(6, P).T.copy()
    fb2 = float(np.asarray(inputs["fb2"]).reshape(-1)[0])

    for dev in range(ndev):
        lo = dev * NPD
        g_lo = int(batch[lo])
        g_hi = int(batch[lo + NPD - 1])
        assert g_hi - g_lo + 1 <= P

        idxw = np.zeros((NT, P, nchunk // 4, 32), np.int16)
        dstloc = np.full((NT, P, nchunk), -1.0, np.float32)
        dstrow = np.full((NT, NE), -1.0, np.float32)
        for t in range(NT):
            gt = dev * NT + t
            s, e = tile_start[gt], tile_start[gt + 1]
            cnt = e - s
            sp = np.zeros(NE, np.int64)
            sp[:cnt] = src[s:e]
            dl = np.full(NE, -1.0, np.float32)
            dl[:cnt] = (dst[s:e] % P).astype(np.float32)
            for gi in range(nchunk // 4):
                idxw[t, :, gi, :] = wrap_idx(sp[gi * 512:(gi + 1) * 512].astype(np.int16))
            dstloc[t] = dl.reshape(nchunk, P).T
            dstrow[t] = dl

        bloc = (batch[lo:lo + NPD].reshape(NT, P).T - g_lo).astype(np.float32)
        poolidx = np.arange(P, dtype=np.int32) + g_lo
        poolidx[poolidx > g_hi] = g
        m = {
            "xT": x[lo:lo + NPD].T.astype(BF),
            "att": att_cols.astype(BF),
            "idxw": idxw,
            "dstloc": dstloc,
            "dstrow": dstrow.astype(BF),
            "bloc": bloc,
            "poolidx": poolidx[:, None],
            "fW1p": fW1p, "fb1p": fb1, "fW2p": fW2p,
            **consts, **wmats,
        }
        in_maps.append(m)
    return in_maps, fb2


def kernel_impl(inputs, trace=False, trace_kwargs=None):
    cfg = Cfg(n=16384, g=256, ndev=8, nchunk=0)
    in_maps, fb2 = preprocess(inputs, cfg)
    nc = build_program(cfg, fb2)
    res = run_bass_kernel_spmd(nc, in_maps, core_ids=list(range(cfg.ndev)),
                               trace=trace, **(trace_kwargs or {}))
    y = np.asarray(res.results[0]["y"], np.float32).reshape(cfg.g, 1)
    return y + fb2, res


def kernel(**inputs) -> np.ndarray:
    y, _ = kernel_impl(inputs)
    return y



# revision 11
# speedup vs baseline: 1.9314x; 1.9314x over previous
"""Trainium2 Bass kernel for 3-layer GATv2 + sum-pool + MLP (nn_GAT_56977036148745).

Strategy (8 NeuronCores, SPMD):
  - Nodes sharded into 8 contiguous slabs of 2048 (dst-sharding). Each core owns
    all edges whose destination lands in its slab (edges sorted by dst on host).
  - Per layer: every core builds its slab of the gather table T = h @ Wl (+bias
    row), an AllGather assembles the full [N, 256] bf16 table in each core's
    DRAM. Edge phase gathers source rows twice per 128-dst tile (channel-major
    via transposing dma_gather for the logit path, edge-major for aggregation).
  - Logits: m = xl[src] + xr[dst] (PE one-hot broadcast + identity-add into
    PSUM), t = LeakyRelu(m) on ACT, per-edge logit = att . t via tiny PE
    matmuls.  exp on ACT (no max subtraction needed; logits are O(5)).
  - Aggregation: per 128-edge chunk, M[e,d] = (dst_e == d) * exp(logit_e) built
    by one DVE tensor_scalar (iota/is_equal/mult), then PSUM-accumulated
    seg-matmuls give numerator and denominator per dst tile.
  - Sum-pool accumulated per layer with graph-one-hot matmuls; partial pools
    scattered into a [G+1, 772] bounce and AllReduced (graphs may straddle core
    boundaries). MLP runs redundantly on every core.
Host preprocessing (sorting, padding, index wrapping, weight folding) is not
part of the measured device time.
"""

import sys

for _p in ("/opt/trn_rl_repo", "/root/.axon_site/_ro/trn_rl_repo"):
    if _p not in sys.path:
        sys.path.append(_p)

import numpy as np
import ml_dtypes

try:  # NTFF profiling hook shim (image's antenv lacks axon_hooks)
    import antenv.axon_hooks  # noqa: F401
except ImportError:
    import types as _types

    try:
        import trn_agent_boot.trn_boot as _tb
        _ntff_hook = _tb._ntff_profile_via_ctypes("/opt/axon/libaxon_pjrt.so")
    except Exception:
        _ntff_hook = None
    _m = _types.ModuleType("antenv.axon_hooks")
    _m.get_axon_ntff_profile_hook = lambda: _ntff_hook
    _m.set_axon_ntff_profile_hook = lambda h: None
    sys.modules["antenv.axon_hooks"] = _m

import concourse.bacc as bacc
import concourse.bass as bass
import concourse.mybir as mybir
import concourse.tile as tile
from concourse.bass import IndirectOffsetOnAxis
from concourse.bass_utils import run_bass_kernel_spmd

BF = ml_dtypes.bfloat16
F32 = mybir.dt.float32
BF16 = mybir.dt.bfloat16
I16 = mybir.dt.int16
I32 = mybir.dt.int32

P = 128          # partitions / dst-tile size / edge-chunk size
H = 2            # heads
C = 128          # channels per head
D = H * C        # 256
AF = mybir.ActivationFunctionType
ALU = mybir.AluOpType
NEG_SLOPE = 0.2


class Cfg:
    def __init__(self, n, g, ndev, nchunk, in_ch=128):
        self.n = n                  # total nodes
        self.g = g                  # graphs
        self.ndev = ndev
        self.nchunk = nchunk        # edge chunks (of 128) per dst tile, mult of 4
        self.in_ch = in_ch          # layer-1 input channels
        self.npd = n // ndev        # nodes per device
        self.nt = self.npd // P     # dst/node tiles per device
        self.pda = 772              # padded pool dim (768 + cnt + pad)


# ----------------------------------------------------------------------------
# device program
# ----------------------------------------------------------------------------

def build_program(cfg: Cfg, fb2: float):
    nc = bacc.Bacc("TRN2", target_bir_lowering=False, debug=False,
                   num_devices=cfg.ndev)
    NT, NC, G, NPD = cfg.nt, cfg.nchunk, cfg.g, cfg.npd
    NE = NC * P                      # padded edges per dst tile
    KC1 = cfg.in_ch // P             # layer-1 K chunks (1)
    PDA = cfg.pda

    dt = nc.dram_tensor
    xT_d = dt("xT", [cfg.in_ch, NPD], BF16, kind="ExternalInput")
    wl_d = [dt(f"wl{l}", [cfg.in_ch if l == 0 else D, D], BF16, kind="ExternalInput") for l in range(3)]
    wr_d = [dt(f"wr{l}", [cfg.in_ch if l == 0 else D, D], BF16, kind="ExternalInput") for l in range(3)]
    rl_d = [None] + [dt(f"rl{l}", [1, D], BF16, kind="ExternalInput") for l in (1, 2)]
    rr_d = [None] + [dt(f"rr{l}", [1, D], BF16, kind="ExternalInput") for l in (1, 2)]
    att_d = dt("att", [C, 2 * 3], BF16, kind="ExternalInput")
    idxw_d = dt("idxw", [NT, P, NC * 8], I16, kind="ExternalInput")
    dstloc_d = dt("dstloc", [NT, P, NC], F32, kind="ExternalInput")
    dstrow_d = dt("dstrow", [NT, NE], BF16, kind="ExternalInput")
    bloc_d = dt("bloc", [P, NT], F32, kind="ExternalInput")
    poolidx_d = dt("poolidx", [P, 1], I32, kind="ExternalInput")
    fW1_d = dt("fW1p", [PDA, 768], F32, kind="ExternalInput")
    fb1_d = dt("fb1p", [P, 6], F32, kind="ExternalInput")
    fW2_d = dt("fW2p", [P, 6], F32, kind="ExternalInput")
    identbf_d = dt("identbf", [P, P], BF16, kind="ExternalInput")
    identf_d = dt("identf", [P, P], F32, kind="ExternalInput")
    iotarow2_d = dt("iotarow2", [P, 2 * P], BF16, kind="ExternalInput")
    iotacol_d = dt("iotacol", [P, 1], F32, kind="ExternalInput")
    ones1p_d = dt("ones1p", [1, P], BF16, kind="ExternalInput")
    onescol_d = dt("onescol", [P, 1], BF16, kind="ExternalInput")
    y_d = dt("y", [1, G], F32, kind="ExternalOutput")

    rg = [list(range(cfg.ndev))]

    with tile.TileContext(nc) as tc:
        with (
            tc.tile_pool(name="persist", bufs=1) as pp,
            tc.tile_pool(name="dram", bufs=2, space="DRAM") as dram,
            tc.tile_pool(name="gath", bufs=2) as gpool,
            tc.tile_pool(name="work", bufs=2) as wpool,
            tc.tile_pool(name="mtile", bufs=3) as mpool,
            tc.tile_pool(name="pm", bufs=1, space="PSUM") as pmpool,
            tc.tile_pool(name="pseg", bufs=2, space="PSUM") as psegpool,
            tc.tile_pool(name="pdst", bufs=1, space="PSUM") as pdstpool,
            tc.tile_pool(name="plog", bufs=1, space="PSUM") as plogpool,
            tc.tile_pool(name="pmisc", bufs=1, space="PSUM") as pmiscpool,
        ):
            # ---- persistent SBUF ----
            identbf = pp.tile([P, P], BF16, tag="identbf")
            identf = pp.tile([P, P], F32, tag="identf")
            iotarow2 = pp.tile([P, 2, P], BF16, tag="iotarow2")
            iotacol = pp.tile([P, 1], F32, tag="iotacol")
            ones1p = pp.tile([1, P], BF16, tag="ones1p")
            onescol = pp.tile([P, 1], BF16, tag="onescol")
            att_sb = pp.tile([C, 6], BF16, tag="att")
            xT_sb = pp.tile([cfg.in_ch, NPD], BF16, tag="xT")
            hT_sb = pp.tile([P, 2, NPD], BF16, tag="hT")
            xr_sb = pp.tile([P, NT, D], BF16, tag="xr")
            pool_sb = pp.tile([P, PDA], F32, tag="pool")
            bloc_sb = pp.tile([P, NT], F32, tag="bloc")
            poolidx_sb = pp.tile([P, 1], I32, tag="poolidx")
            wl_sb = [pp.tile([P, (cfg.in_ch if l == 0 else D) // P, D], BF16, name=f"wl{l}", tag=f"wl{l}") for l in range(3)]
            wr_sb = [pp.tile([P, (cfg.in_ch if l == 0 else D) // P, D], BF16, name=f"wr{l}", tag=f"wr{l}") for l in range(3)]
            rl_sb = [None, pp.tile([1, D], BF16, name="rl1", tag="rl1"), pp.tile([1, D], BF16, name="rl2", tag="rl2")]
            rr_sb = [None, pp.tile([1, D], BF16, name="rr1", tag="rr1"), pp.tile([1, D], BF16, name="rr2", tag="rr2")]
            zero_sb = pp.tile([P, PDA], F32, tag="zero")

            for sb, d in ((identbf, identbf_d), (identf, identf_d),
                          (iotarow2, iotarow2_d), (iotacol, iotacol_d),
                          (ones1p, ones1p_d), (onescol, onescol_d),
                          (att_sb, att_d), (xT_sb, xT_d), (bloc_sb, bloc_d),
                          (poolidx_sb, poolidx_d)):
                nc.sync.dma_start(sb[:], d[:])
            for l in range(3):
                kc = (cfg.in_ch if l == 0 else D) // P
                nc.sync.dma_start(wl_sb[l][:], wl_d[l].ap().rearrange("(k p) d -> p k d", p=P))
                nc.sync.dma_start(wr_sb[l][:], wr_d[l].ap().rearrange("(k p) d -> p k d", p=P))
                if l > 0:
                    nc.sync.dma_start(rl_sb[l][:], rl_d[l][:])
                    nc.sync.dma_start(rr_sb[l][:], rr_d[l][:])
            nc.vector.memset(pool_sb[:], 0.0)
            nc.vector.memset(zero_sb[:], 0.0)

            # pool bounce (zeroed before scatter)
            poolb_in = dram.tile([G + 1, PDA], F32, tag="poolb_in")
            poolb_out = dram.tile([G + 1, PDA], F32, tag="poolb_out")
            for r0 in range(0, G + 1, P):
                rows = min(P, G + 1 - r0)
                nc.sync.dma_start(poolb_in[r0:r0 + rows, :], zero_sb[:rows, :])

            # ------------------------------------------------------------------
            for l in range(3):
                kcs = KC1 if l == 0 else 2
                src_sb = xT_sb if l == 0 else hT_sb

                def src_lhsT(kc, nt):
                    if l == 0:
                        return src_sb[:, nt * P:(nt + 1) * P]
                    return src_sb[:, kc, nt * P:(nt + 1) * P]

                # ---- table slab + XR slab ----
                slab = dram.tile([NPD, D], BF16, tag="slab")
                Tfull = dram.tile([cfg.n, D], BF16, tag="Tfull")
                for nt in range(NT):
                    ptab = pmiscpool.tile([P, D], F32, tag="ptab")
                    for kc in range(kcs):
                        nc.tensor.matmul(ptab[:], src_lhsT(kc, nt), wl_sb[l][:, kc, :],
                                         start=(kc == 0), stop=(kc == kcs - 1 and l == 0))
                    if l > 0:
                        nc.tensor.matmul(ptab[:], ones1p[:], rl_sb[l][:], start=False, stop=True)
                    tab = wpool.tile([P, D], BF16, tag="tab")
                    nc.vector.tensor_copy(tab[:], ptab[:])
                    nc.sync.dma_start(slab[nt * P:(nt + 1) * P, :], tab[:])

                    pxr = pmiscpool.tile([P, D], F32, tag="ptab")
                    for kc in range(kcs):
                        nc.tensor.matmul(pxr[:], src_lhsT(kc, nt), wr_sb[l][:, kc, :],
                                         start=(kc == 0), stop=(kc == kcs - 1 and l == 0))
                    if l > 0:
                        nc.tensor.matmul(pxr[:], ones1p[:], rr_sb[l][:], start=False, stop=True)
                    nc.vector.tensor_copy(xr_sb[:, nt, :], pxr[:])

                nc.gpsimd.collective_compute(
                    "AllGather", ALU.bypass, replica_groups=rg,
                    ins=[slab.opt()], outs=[Tfull.opt()],
                )

                # ---- edge phase ----
                ppool = pmiscpool.tile([P, D + 1], F32, tag="ppool")
                for t in range(NT):
                    idx_sb = wpool.tile([P, NC * 8], I16, tag="idx")
                    nc.sync.dma_start(idx_sb[:], idxw_d[t])
                    dstrow = wpool.tile([1, NE], BF16, tag="dstrow")
                    nc.sync.dma_start(dstrow[:], dstrow_d[t:t + 1, :])
                    dstloc = wpool.tile([P, NC], F32, tag="dstloc")
                    nc.sync.dma_start(dstloc[:], dstloc_d[t])

                    # split to keep each call's descriptors within the SWDGE
                    # ring carveout (16KB/partition; 4608 descs would overflow)
                    xlE = gpool.tile([P, NC, D], BF16, tag="xlE")
                    GS = 4  # chunks per gather call
                    for g0 in range(0, NC, GS):
                        gn = min(GS, NC - g0)
                        nc.gpsimd.dma_gather(
                            xlE[:, g0:g0 + gn, :], Tfull[:],
                            idx_sb[:, g0 * 8:(g0 + gn) * 8], gn * P, gn * P, D)

                    pseg = psegpool.tile([P, 2 * (C + 1)], F32, tag="pseg")
                    plog = plogpool.tile([P, 2 * NC], F32, tag="plog")

                    for g in range(NC // 4):
                        e0 = g * 512
                        pdst = pdstpool.tile([P, 512], F32, tag="pdst")
                        nc.tensor.matmul(pdst[:], ones1p[:], dstrow[:, e0:e0 + 512],
                                         start=True, stop=True)
                        onehot = wpool.tile([P, 512], BF16, tag="onehot")
                        nc.vector.tensor_scalar(onehot[:], pdst[:], iotacol[:], None,
                                                ALU.is_equal)
                        pm = pmpool.tile([P, H, 512], F32, tag="pm")
                        for h in range(H):
                            nc.tensor.matmul(pm[:, h, :],
                                             xr_sb[:, t, h * C:(h + 1) * C],
                                             onehot[:], start=True, stop=False)
                            # += xl[src]^T via PE transpose of the edge-major
                            # gather (replaces the transposed dma_gather)
                            for sub in range(4):
                                nc.tensor.matmul(
                                    pm[:, h, sub * P:(sub + 1) * P],
                                    xlE[:, 4 * g + sub, h * C:(h + 1) * C],
                                    identbf[:], start=False, stop=True)
                        tsb = wpool.tile([P, H, 512], BF16, tag="tsb")
                        nc.scalar.activation(tsb[:], pm[:], AF.Prelu, alpha=NEG_SLOPE)
                        for h in range(H):
                            for sub in range(4):
                                k = g * 4 + sub
                                nc.tensor.matmul(
                                    plog[:, 2 * k + h:2 * k + h + 1],
                                    tsb[:, h, sub * P:(sub + 1) * P],
                                    att_sb[:, l * 2 + h:l * 2 + h + 1],
                                    start=True, stop=True)

                    ev = wpool.tile([P, 2 * NC], F32, tag="ev")
                    nc.scalar.activation(ev[:], plog[:], AF.Exp)

                    for k in range(NC):
                        # both heads' dst-onehot, scaled by exp(logit), in one op:
                        # Mt2[:, h, d] = (iota[d] == dstloc_e) * ev[e, h]
                        Mt2 = mpool.tile([P, 2, P], BF16, tag="Mt")
                        evs = ev[:, 2 * k:2 * k + 2]
                        ev_bc = bass.AP(evs.tensor, evs.offset,
                                        [list(p) for p in evs.ap] + [[0, P]])
                        nc.vector.scalar_tensor_tensor(
                            Mt2[:], iotarow2[:], dstloc[:, k:k + 1], ev_bc,
                            ALU.is_equal, ALU.mult)
                        for h in range(H):
                            base = h * (C + 1)
                            nc.tensor.matmul(pseg[:, base:base + C], Mt2[:, h, :],
                                             xlE[:, k, h * C:(h + 1) * C],
                                             start=(k == 0 and h == 0), stop=False)
                            nc.tensor.matmul(pseg[:, base + C:base + C + 1], Mt2[:, h, :],
                                             onescol[:], start=False,
                                             stop=(k == NC - 1 and h == 1))

                    rec = wpool.tile([P, 2], F32, tag="rec")
                    hst = wpool.tile([P, D], BF16, tag="hst")
                    for h in range(H):
                        base = h * (C + 1)
                        nc.vector.reciprocal(rec[:, h:h + 1], pseg[:, base + C:base + C + 1])
                        nc.scalar.mul(hst[:, h * C:(h + 1) * C],
                                      pseg[:, base:base + C], rec[:, h:h + 1])

                    # pooling
                    Gt = wpool.tile([P, P], BF16, tag="Gt")
                    nc.vector.tensor_scalar(Gt[:], iotarow2[:, 0, :], bloc_sb[:, t:t + 1],
                                            None, ALU.is_equal)
                    nc.tensor.matmul(ppool[:, :D], Gt[:], hst[:],
                                     start=(t == 0), stop=(t == NT - 1 and l != 0))
                    if l == 0:
                        nc.tensor.matmul(ppool[:, D:D + 1], Gt[:], onescol[:],
                                         start=False, stop=(t == NT - 1))

                    # transpose h for next layer's table build
                    if l < 2:
                        for h in range(H):
                            ptr = pmiscpool.tile([P, P], BF16, tag="ptab")
                            nc.tensor.transpose(ptr[:], hst[:, h * C:(h + 1) * C],
                                                identbf[:])
                            nc.vector.tensor_copy(hT_sb[:, h, t * P:(t + 1) * P], ptr[:])

                nc.vector.tensor_copy(pool_sb[:, l * D:(l + 1) * D], ppool[:, :D])
                if l == 0:
                    nc.vector.tensor_copy(pool_sb[:, 768:769], ppool[:, D:D + 1])

            # ------------------------------------------------------------------
            # pooling allreduce + MLP
            nc.gpsimd.indirect_dma_start(
                out=poolb_in[:],
                out_offset=IndirectOffsetOnAxis(ap=poolidx_sb[:, :1], axis=0),
                in_=pool_sb[:],
                in_offset=None,
            )
            nc.gpsimd.collective_compute(
                "AllReduce", ALU.add, replica_groups=rg,
                ins=[poolb_in.opt()], outs=[poolb_out.opt()],
            )

            fW1_sb = [pp.tile([P, 768], F32, name=f"fW1_{kc}", tag=f"fW1_{kc}") for kc in range(7)]
            for kc in range(7):
                kr = min(P, PDA - kc * P)
                nc.sync.dma_start(fW1_sb[kc][:kr, :], fW1_d[kc * P:kc * P + kr, :])
            fb1_sb = pp.tile([P, 6], F32, tag="fb1")
            nc.sync.dma_start(fb1_sb[:], fb1_d[:])
            fW2_sb = pp.tile([P, 6], F32, tag="fW2")
            nc.sync.dma_start(fW2_sb[:], fW2_d[:])

            poolT = [pp.tile([P, max(G, P)], F32, name=f"poolT_{kc}", tag=f"poolT_{kc}") for kc in range(7)]
            for rt in range(0, G, P):
                rows = min(P, G - rt)
                prow = wpool.tile([P, PDA], F32, tag="prow")
                nc.sync.dma_start(prow[:rows, :], poolb_out[rt:rt + rows, :])
                for cb in range(7):
                    w = min(P, PDA - cb * P)
                    ptr2 = pmiscpool.tile([P, P], F32, tag="ptab")
                    nc.tensor.transpose(ptr2[:w, :rows], prow[:rows, cb * P:cb * P + w],
                                        identf[:rows, :rows])
                    nc.vector.tensor_copy(poolT[cb][:w, rt:rt + rows], ptr2[:w, :rows])

            h1_sb = [pp.tile([P, max(G, P)], F32, name=f"h1_{mo}", tag=f"h1_{mo}") for mo in range(6)]
            for mo in range(6):
                ph1 = pmiscpool.tile([P, max(G, P)], F32, tag="ptab")
                for kc in range(7):
                    kr = min(P, PDA - kc * P)
                    nc.tensor.matmul(ph1[:, :G], fW1_sb[kc][:kr, mo * P:(mo + 1) * P],
                                     poolT[kc][:kr, :G], start=(kc == 0), stop=(kc == 6))
                nc.scalar.activation(h1_sb[mo][:, :G], ph1[:, :G], AF.Relu,
                                     bias=fb1_sb[:, mo:mo + 1])

            py = pmiscpool.tile([1, max(G, P)], F32, tag="ppool")
            for mo in range(6):
                nc.tensor.matmul(py[:, :G], fW2_sb[:, mo:mo + 1], h1_sb[mo][:, :G],
                                 start=(mo == 0), stop=(mo == 5))
            ysb = wpool.tile([1, max(G, P)], F32, tag="ysb")
            nc.vector.tensor_copy(ysb[:, :G], py[:, :G])
            nc.sync.dma_start(y_d[:], ysb[:1, :G])

    nc.compile()
    return nc


# ----------------------------------------------------------------------------
# host preprocessing
# ----------------------------------------------------------------------------

def preprocess(inputs: dict, cfg: Cfg):
    n, g, ndev = cfg.n, cfg.g, cfg.ndev
    NPD, NT = cfg.npd, cfg.nt

    x = np.asarray(inputs["x"], np.float32)
    ei = np.asarray(inputs["edge_index"]).astype(np.int64)
    batch = np.asarray(inputs["batch"]).astype(np.int64)

    src = np.concatenate([ei[0], np.arange(n)])
    dst = np.concatenate([ei[1], np.arange(n)])
    order = np.argsort(dst, kind="stable")
    src, dst = src[order], dst[order]

    # per (dev, tile) edge lists
    tile_of = dst // P              # global dst tile id (NT per device)
    counts = np.bincount(tile_of, minlength=(n // P))
    nchunk = int(np.ceil(counts.max() / P))
    nchunk = ((nchunk + 3) // 4) * 4
    cfg.nchunk = nchunk
    NE = nchunk * P

    tile_start = np.zeros(n // P + 1, np.int64)
    np.cumsum(counts, out=tile_start[1:])

    def wrap_idx(a):  # [NE] int16 -> [128, NE // 16]
        w = a.reshape(-1, 16).T.copy()          # [16, NE // 16]
        return np.tile(w, (8, 1))               # [128, NE // 16]

    in_maps = []
    consts = {
        "identbf": np.eye(P, dtype=BF),
        "identf": np.eye(P, dtype=np.float32),
        "iotarow2": np.tile(np.arange(P, dtype=BF)[None, :], (P, 2)),
        "iotacol": np.arange(P, dtype=np.float32)[:, None],
        "ones1p": np.ones((1, P), BF),
        "onescol": np.ones((P, 1), BF),
    }
    att_all = np.stack([np.asarray(inputs[f"att{l+1}"], np.float32) for l in range(3)])  # [3, H, C]
    att_cols = np.zeros((C, 6), np.float32)
    for l in range(3):
        for h in range(H):
            att_cols[:, l * 2 + h] = att_all[l, h]

    b = [np.asarray(inputs[f"b{l+1}"], np.float32) for l in range(3)]
    wmats = {}
    for l in range(3):
        wmats[f"wl{l}"] = np.asarray(inputs[f"Wl{l+1}"], np.float32).astype(BF)
        wmats[f"wr{l}"] = np.asarray(inputs[f"Wr{l+1}"], np.float32).astype(BF)
    for l in (1, 2):
        wmats[f"rl{l}"] = (b[l - 1] @ np.asarray(inputs[f"Wl{l+1}"], np.float32))[None, :].astype(BF)
        wmats[f"rr{l}"] = (b[l - 1] @ np.asarray(inputs[f"Wr{l+1}"], np.float32))[None, :].astype(BF)

    b_all = np.concatenate(b)
    fW1 = np.asarray(inputs["fW1"], np.float32)
    fW1p = np.zeros((cfg.pda, 768), np.float32)
    fW1p[:768] = fW1
    fW1p[768] = b_all @ fW1
    fb1 = np.asarray(inputs["fb1"], np.float32).reshape(6, P).T.copy()
    fW2p = np.asarray(inputs["fW2"], np.float32).reshape(6, P).T.copy()
    fb2 = float(np.asarray(inputs["fb2"]).reshape(-1)[0])

    for dev in range(ndev):
        lo = dev * NPD
        g_lo = int(batch[lo])
        g_hi = int(batch[lo + NPD - 1])
        assert g_hi - g_lo + 1 <= P

        idxw = np.zeros((NT, P, nchunk * 8), np.int16)
        dstloc = np.full((NT, P, nchunk), -1.0, np.float32)
        dstrow = np.full((NT, NE), -1.0, np.float32)
        for t in range(NT):
            gt = dev * NT + t
            s, e = tile_start[gt], tile_start[gt + 1]
            cnt = e - s
            sp = np.zeros(NE, np.int64)
            sp[:cnt] = src[s:e]
            dl = np.full(NE, -1.0, np.float32)
            dl[:cnt] = (dst[s:e] % P).astype(np.float32)
            idxw[t] = wrap_idx(sp.astype(np.int16))
            dstloc[t] = dl.reshape(nchunk, P).T
            dstrow[t] = dl

        bloc = (batch[lo:lo + NPD].reshape(NT, P).T - g_lo).astype(np.float32)
        poolidx = np.arange(P, dtype=np.int32) + g_lo
        poolidx[poolidx > g_hi] = g
        m = {
            "xT": x[lo:lo + NPD].T.astype(BF),
            "att": att_cols.astype(BF),
            "idxw": idxw,
            "dstloc": dstloc,
            "dstrow": dstrow.astype(BF),
            "bloc": bloc,
            "poolidx": poolidx[:, None],
            "fW1p": fW1p, "fb1p": fb1, "fW2p": fW2p,
            **consts, **wmats,
        }
        in_maps.append(m)
    return in_maps, fb2


def kernel_impl(inputs, trace=False, trace_kwargs=None):
    cfg = Cfg(n=16384, g=256, ndev=8, nchunk=0)
    in_maps, fb2 = preprocess(inputs, cfg)
    nc = build_program(cfg, fb2)
    res = run_bass_kernel_spmd(nc, in_maps, core_ids=list(range(cfg.ndev)),
                               trace=trace, **(trace_kwargs or {}))
    y = np.asarray(res.results[0]["y"], np.float32).reshape(cfg.g, 1)
    return y + fb2, res


def kernel(**inputs) -> np.ndarray:
    y, _ = kernel_impl(inputs)
    return y



# revision 29
# speedup vs baseline: 1.9715x; 1.0207x over previous
"""Trainium2 Bass kernel for 3-layer GATv2 + sum-pool + MLP (nn_GAT_56977036148745).

Strategy (8 NeuronCores, SPMD):
  - Nodes sharded into 8 contiguous slabs of 2048 (dst-sharding). Each core owns
    all edges whose destination lands in its slab (edges sorted by dst on host).
  - Per layer: each core builds its slab of the gather table with rows packed
    [xl_h0 | 1 | xl_h1 | 1 | pad->384] (the 1s provide softmax denominators for
    free); an AllGather assembles the full [N, 384] bf16 table in every core's
    DRAM.  The slab rows for layer l+1 are built inline inside layer l's edge
    loop (right after each tile's output transpose) so only the AllGather
    itself sits on the layer boundary.
  - Edge phase: ONE edge-major dma_gather per dst tile (4608 idx,
    single_packet=False -- with single-packet mode the 64-desc/engine packet
    cap limits calls to 1024 idx and larger calls hang).  The channel-major copy needed by the logit path comes from PE
    transposes (xlE^T via identity matmul) accumulated into the same PSUM as
    the xr one-hot broadcast; LeakyRelu on ACT, per-edge logit = att . t via
    small PE matmuls, exp on ACT.
  - Aggregation: per 128-edge chunk, both heads' scaled one-hots
    Mt2[e,(h,d)] = (dst_e == d) * exp(logit_e,h) are built by ONE DVE
    scalar_tensor_tensor (stride-0 broadcast AP on ev), then one 129-col
    PSUM-accumulated matmul per head yields numerator and denominator.
    PSUM accumulation start/stop must bracket the whole pseg group ONCE
    (start=True clears the whole bank, not just the addressed columns).
  - Sum-pool accumulated per layer with graph-one-hot matmuls; partial pools
    scattered into a [G+1, 772] bounce and AllReduced (graphs may straddle core
    boundaries). MLP runs redundantly on every core.
Host preprocessing (sorting, padding, index wrapping, weight folding) is not
part of the measured device time.
"""

import sys

for _p in ("/opt/trn_rl_repo", "/root/.axon_site/_ro/trn_rl_repo"):
    if _p not in sys.path:
        sys.path.append(_p)

import numpy as np
import ml_dtypes

try:  # NTFF profiling hook shim (image's antenv lacks axon_hooks)
    import antenv.axon_hooks  # noqa: F401
except ImportError:
    import types as _types

    try:
        import trn_agent_boot.trn_boot as _tb
        _ntff_hook = _tb._ntff_profile_via_ctypes("/opt/axon/libaxon_pjrt.so")
    except Exception:
        _ntff_hook = None
    _m = _types.ModuleType("antenv.axon_hooks")
    _m.get_axon_ntff_profile_hook = lambda: _ntff_hook
    _m.set_axon_ntff_profile_hook = lambda h: None
    sys.modules["antenv.axon_hooks"] = _m

import concourse.bacc as bacc
import concourse.bass as bass
import concourse.mybir as mybir
import concourse.tile as tile
from concourse.bass import IndirectOffsetOnAxis
from concourse.bass_utils import run_bass_kernel_spmd

BF = ml_dtypes.bfloat16
F32 = mybir.dt.float32
BF16 = mybir.dt.bfloat16
I16 = mybir.dt.int16
I32 = mybir.dt.int32

P = 128          # partitions / dst-tile size / edge-chunk size
H = 2            # heads
C = 128          # channels per head
D = H * C        # 256
TW = 384         # packed table row: [xl_h0 | 1 | xl_h1 | 1 | pad] (mult of 128)
AF = mybir.ActivationFunctionType
ALU = mybir.AluOpType
NEG_SLOPE = 0.2


class Cfg:
    def __init__(self, n, g, ndev, nchunk, in_ch=128):
        self.n = n                  # total nodes
        self.g = g                  # graphs
        self.ndev = ndev
        self.nchunk = nchunk        # edge chunks (of 128) per dst tile, mult of 4
        self.in_ch = in_ch          # layer-1 input channels
        self.npd = n // ndev        # nodes per device
        self.nt = self.npd // P     # dst/node tiles per device
        self.pda = 772              # padded pool dim (768 + cnt + pad)


# ----------------------------------------------------------------------------
# device program
# ----------------------------------------------------------------------------

def build_program(cfg: Cfg, fb2: float):
    nc = bacc.Bacc("TRN2", target_bir_lowering=False, debug=False,
                   num_devices=cfg.ndev)
    NT, NC, G, NPD = cfg.nt, cfg.nchunk, cfg.g, cfg.npd
    NE = NC * P                      # padded edges per dst tile
    KC1 = cfg.in_ch // P             # layer-1 K chunks (1)
    PDA = cfg.pda

    dt = nc.dram_tensor
    xT_d = dt("xT", [cfg.in_ch, NPD], BF16, kind="ExternalInput")
    wl_d = [dt(f"wl{l}", [cfg.in_ch if l == 0 else D, D], BF16, kind="ExternalInput") for l in range(3)]
    wr_d = [dt(f"wr{l}", [cfg.in_ch if l == 0 else D, D], BF16, kind="ExternalInput") for l in range(3)]
    rl_d = [None] + [dt(f"rl{l}", [1, D], BF16, kind="ExternalInput") for l in (1, 2)]
    rr_d = [None] + [dt(f"rr{l}", [1, D], BF16, kind="ExternalInput") for l in (1, 2)]
    att_d = dt("att", [C, 2 * 3], BF16, kind="ExternalInput")
    idxw_d = dt("idxw", [NT, P, NC * 8], I16, kind="ExternalInput")
    dstloc_d = dt("dstloc", [NT, P, NC], F32, kind="ExternalInput")
    dstrow_d = dt("dstrow", [NT, NE], BF16, kind="ExternalInput")
    bloc_d = dt("bloc", [P, NT], F32, kind="ExternalInput")
    poolidx_d = dt("poolidx", [P, 1], I32, kind="ExternalInput")
    fW1_d = dt("fW1p", [PDA, 768], F32, kind="ExternalInput")
    fb1_d = dt("fb1p", [P, 6], F32, kind="ExternalInput")
    fW2_d = dt("fW2p", [P, 6], F32, kind="ExternalInput")
    identbf_d = dt("identbf", [P, P], BF16, kind="ExternalInput")
    identf_d = dt("identf", [P, P], F32, kind="ExternalInput")
    iotarow2_d = dt("iotarow2", [P, 2 * P], BF16, kind="ExternalInput")
    iotacol_d = dt("iotacol", [P, 1], F32, kind="ExternalInput")
    ones1p_d = dt("ones1p", [1, P], BF16, kind="ExternalInput")
    onescol_d = dt("onescol", [P, 1], BF16, kind="ExternalInput")
    onesrow1_d = dt("onesrow1", [1, 1], BF16, kind="ExternalInput")
    y_d = dt("y", [1, G], F32, kind="ExternalOutput")

    rg = [list(range(cfg.ndev))]

    with tile.TileContext(nc) as tc:
        with (
            tc.tile_pool(name="persist", bufs=1) as pp,
            tc.tile_pool(name="dram", bufs=2, space="DRAM") as dram,
            tc.tile_pool(name="gath", bufs=3) as gpool,
            tc.tile_pool(name="work", bufs=2) as wpool,
            tc.tile_pool(name="mtile", bufs=3) as mpool,
            tc.tile_pool(name="pm", bufs=1, space="PSUM") as pmpool,
            tc.tile_pool(name="pseg", bufs=2, space="PSUM") as psegpool,
            tc.tile_pool(name="pdst", bufs=1, space="PSUM") as pdstpool,
            tc.tile_pool(name="plog", bufs=1, space="PSUM") as plogpool,
            tc.tile_pool(name="pmisc", bufs=1, space="PSUM") as pmiscpool,
        ):
            # ---- persistent SBUF ----
            identbf = pp.tile([P, P], BF16, tag="identbf")
            identf = pp.tile([P, P], F32, tag="identf")
            iotarow2 = pp.tile([P, 2, P], BF16, tag="iotarow2")
            iotacol = pp.tile([P, 1], F32, tag="iotacol")
            ones1p = pp.tile([1, P], BF16, tag="ones1p")
            onescol = pp.tile([P, 1], BF16, tag="onescol")
            onesrow1 = pp.tile([1, 1], BF16, tag="onesrow1")
            att_sb = pp.tile([C, 6], BF16, tag="att")
            xT_sb = pp.tile([cfg.in_ch, NPD], BF16, tag="xT")
            hT_sb = pp.tile([P, 2, NPD], BF16, tag="hT")
            xr_sb = pp.tile([P, NT, D], BF16, tag="xr")
            pool_sb = pp.tile([P, PDA], F32, tag="pool")
            bloc_sb = pp.tile([P, NT], F32, tag="bloc")
            poolidx_sb = pp.tile([P, 1], I32, tag="poolidx")
            wl_sb = [pp.tile([P, (cfg.in_ch if l == 0 else D) // P, D], BF16, name=f"wl{l}", tag=f"wl{l}") for l in range(3)]
            wr_sb = [pp.tile([P, (cfg.in_ch if l == 0 else D) // P, D], BF16, name=f"wr{l}", tag=f"wr{l}") for l in range(3)]
            rl_sb = [None, pp.tile([1, D], BF16, name="rl1", tag="rl1"), pp.tile([1, D], BF16, name="rl2", tag="rl2")]
            rr_sb = [None, pp.tile([1, D], BF16, name="rr1", tag="rr1"), pp.tile([1, D], BF16, name="rr2", tag="rr2")]
            zero_sb = pp.tile([P, PDA], F32, tag="zero")

            for sb, d in ((identbf, identbf_d), (identf, identf_d),
                          (iotarow2, iotarow2_d), (iotacol, iotacol_d),
                          (ones1p, ones1p_d), (onescol, onescol_d),
                          (onesrow1, onesrow1_d),
                          (att_sb, att_d), (xT_sb, xT_d), (bloc_sb, bloc_d),
                          (poolidx_sb, poolidx_d)):
                nc.sync.dma_start(sb[:], d[:])
            for l in range(3):
                kc = (cfg.in_ch if l == 0 else D) // P
                nc.sync.dma_start(wl_sb[l][:], wl_d[l].ap().rearrange("(k p) d -> p k d", p=P))
                nc.sync.dma_start(wr_sb[l][:], wr_d[l].ap().rearrange("(k p) d -> p k d", p=P))
                if l > 0:
                    nc.sync.dma_start(rl_sb[l][:], rl_d[l][:])
                    nc.sync.dma_start(rr_sb[l][:], rr_d[l][:])
            nc.vector.memset(pool_sb[:], 0.0)
            nc.vector.memset(zero_sb[:], 0.0)

            # pool bounce (zeroed before scatter)
            poolb_in = dram.tile([G + 1, PDA], F32, tag="poolb_in")
            poolb_out = dram.tile([G + 1, PDA], F32, tag="poolb_out")
            for r0 in range(0, G + 1, P):
                rows = min(P, G + 1 - r0)
                nc.sync.dma_start(poolb_in[r0:r0 + rows, :], zero_sb[:rows, :])

            # ------------------------------------------------------------------
            def src_lhsT(l, kc, nt):
                if l == 0:
                    return xT_sb[:, nt * P:(nt + 1) * P]
                return hT_sb[:, kc, nt * P:(nt + 1) * P]

            def build_tile(l, nt, slab):
                # rows packed [xl_h0 | 1 | xl_h1 | 1 | pad]: numerator +
                # denominator of the aggregation in one 129-col matmul/head
                kcs = KC1 if l == 0 else 2
                ptab = pmiscpool.tile([P, 2 * (C + 1)], F32, tag="ptab")
                for h in range(H):
                    hb = h * (C + 1)
                    for kc in range(kcs):
                        nc.tensor.matmul(ptab[:, hb:hb + C], src_lhsT(l, kc, nt),
                                         wl_sb[l][:, kc, h * C:(h + 1) * C],
                                         start=(kc == 0 and h == 0), stop=False)
                    if l > 0:
                        nc.tensor.matmul(ptab[:, hb:hb + C], ones1p[:],
                                         rl_sb[l][:, h * C:(h + 1) * C],
                                         start=False, stop=False)
                    nc.tensor.matmul(ptab[:, hb + C:hb + C + 1], ones1p[:],
                                     onesrow1[:], start=False,
                                     stop=(h == H - 1))
                tab = wpool.tile([P, TW], BF16, tag="tab")
                nc.vector.tensor_copy(tab[:, :2 * (C + 1)], ptab[:])
                nc.sync.dma_start(slab[nt * P:(nt + 1) * P, :], tab[:])

                pxr = pmiscpool.tile([P, D], F32, tag="ptab")
                kcs = KC1 if l == 0 else 2
                for kc in range(kcs):
                    nc.tensor.matmul(pxr[:], src_lhsT(l, kc, nt), wr_sb[l][:, kc, :],
                                     start=(kc == 0), stop=(kc == kcs - 1 and l == 0))
                if l > 0:
                    nc.tensor.matmul(pxr[:], ones1p[:], rr_sb[l][:], start=False, stop=True)
                nc.vector.tensor_copy(xr_sb[:, nt, :], pxr[:])

            slabs = [dram.tile([NPD, TW], BF16, name=f"slab{l}", tag=f"slab{l}") for l in range(3)]
            Tfulls = [dram.tile([cfg.n, TW], BF16, name=f"Tfull{l}", tag=f"Tfull{l}") for l in range(3)]

            # prologue: layer-0 table from the input features
            for nt in range(NT):
                build_tile(0, nt, slabs[0])
            nc.gpsimd.collective_compute(
                "AllGather", ALU.bypass, replica_groups=rg,
                ins=[slabs[0].opt()], outs=[Tfulls[0].opt()],
            )

            for l in range(3):
                Tfull = Tfulls[l]

                # ---- edge phase ----
                ppool = pmiscpool.tile([P, D + 1], F32, tag="ppool")
                for t in range(NT):
                    idx_sb = wpool.tile([P, NC * 8], I16, tag="idx")
                    nc.sync.dma_start(idx_sb[:], idxw_d[t])
                    dstrow = wpool.tile([1, NE], BF16, tag="dstrow")
                    nc.sync.dma_start(dstrow[:], dstrow_d[t:t + 1, :])
                    dstloc = wpool.tile([P, NC], F32, tag="dstloc")
                    nc.sync.dma_start(dstloc[:], dstloc_d[t])

                    # split to keep each call's descriptors within the SWDGE
                    # ring carveout (16KB/partition; 4608 descs would overflow)
                    xlE = gpool.tile([P, NC, TW], BF16, tag="xlE")
                    nc.gpsimd.dma_gather(xlE[:], Tfull[:], idx_sb[:],
                                         NE, NE, TW, single_packet=False)

                    pseg = psegpool.tile([P, 2 * (C + 1)], F32, tag="pseg")
                    plog = plogpool.tile([P, 2 * NC], F32, tag="plog")

                    for g in range(NC // 4):
                        e0 = g * 512
                        pdst = pdstpool.tile([P, 512], F32, tag="pdst")
                        nc.tensor.matmul(pdst[:], ones1p[:], dstrow[:, e0:e0 + 512],
                                         start=True, stop=True)
                        onehot = wpool.tile([P, 512], BF16, tag="onehot")
                        nc.vector.tensor_scalar(onehot[:], pdst[:], iotacol[:], None,
                                                ALU.is_equal)
                        pm = pmpool.tile([P, H, 512], F32, tag="pm")
                        for h in range(H):
                            nc.tensor.matmul(pm[:, h, :],
                                             xr_sb[:, t, h * C:(h + 1) * C],
                                             onehot[:], start=True, stop=False)
                            # += xl[src]^T via PE transpose of the edge-major
                            # gather (replaces the transposed dma_gather)
                            for sub in range(4):
                                nc.tensor.matmul(
                                    pm[:, h, sub * P:(sub + 1) * P],
                                    xlE[:, 4 * g + sub, h * (C + 1):h * (C + 1) + C],
                                    identbf[:], start=False, stop=True)
                        tsb = wpool.tile([P, H, 512], BF16, tag="tsb")
                        nc.scalar.activation(tsb[:], pm[:], AF.Prelu, alpha=NEG_SLOPE)
                        for h in range(H):
                            for sub in range(4):
                                k = g * 4 + sub
                                nc.tensor.matmul(
                                    plog[:, 2 * k + h:2 * k + h + 1],
                                    tsb[:, h, sub * P:(sub + 1) * P],
                                    att_sb[:, l * 2 + h:l * 2 + h + 1],
                                    start=True, stop=True)

                    ev = wpool.tile([P, 2 * NC], F32, tag="ev")
                    nc.scalar.activation(ev[:], plog[:], AF.Exp)

                    for k in range(NC):
                        # both heads' dst-onehot, scaled by exp(logit), in one op:
                        # Mt2[:, h, d] = (iota[d] == dstloc_e) * ev[e, h]
                        Mt2 = mpool.tile([P, 2, P], BF16, tag="Mt")
                        evs = ev[:, 2 * k:2 * k + 2]
                        ev_bc = bass.AP(evs.tensor, evs.offset,
                                        [list(p) for p in evs.ap] + [[0, P]])
                        nc.vector.scalar_tensor_tensor(
                            Mt2[:], iotarow2[:], dstloc[:, k:k + 1], ev_bc,
                            ALU.is_equal, ALU.mult)
                        for h in range(H):
                            base = h * (C + 1)
                            nc.tensor.matmul(pseg[:, base:base + C + 1], Mt2[:, h, :],
                                             xlE[:, k, base:base + C + 1],
                                             start=(k == 0 and h == 0),
                                             stop=(k == NC - 1 and h == 1))

                    rec = wpool.tile([P, 2], F32, tag="rec")
                    hst = wpool.tile([P, D], BF16, tag="hst")
                    for h in range(H):
                        base = h * (C + 1)
                        nc.vector.reciprocal(rec[:, h:h + 1], pseg[:, base + C:base + C + 1])
                        nc.scalar.mul(hst[:, h * C:(h + 1) * C],
                                      pseg[:, base:base + C], rec[:, h:h + 1])

                    # pooling
                    Gt = wpool.tile([P, P], BF16, tag="Gt")
                    nc.vector.tensor_scalar(Gt[:], iotarow2[:, 0, :], bloc_sb[:, t:t + 1],
                                            None, ALU.is_equal)
                    nc.tensor.matmul(ppool[:, :D], Gt[:], hst[:],
                                     start=(t == 0), stop=(t == NT - 1 and l != 0))
                    if l == 0:
                        nc.tensor.matmul(ppool[:, D:D + 1], Gt[:], onescol[:],
                                         start=False, stop=(t == NT - 1))

                    # transpose h, then immediately build this tile's rows of
                    # the next layer's table slab (overlaps the layer boundary)
                    if l < 2:
                        for h in range(H):
                            ptr = pmiscpool.tile([P, P], BF16, tag="ptab")
                            nc.tensor.transpose(ptr[:], hst[:, h * C:(h + 1) * C],
                                                identbf[:])
                            nc.vector.tensor_copy(hT_sb[:, h, t * P:(t + 1) * P], ptr[:])
                        build_tile(l + 1, t, slabs[l + 1])

                nc.vector.tensor_copy(pool_sb[:, l * D:(l + 1) * D], ppool[:, :D])
                if l == 0:
                    nc.vector.tensor_copy(pool_sb[:, 768:769], ppool[:, D:D + 1])
                if l < 2:
                    nc.gpsimd.collective_compute(
                        "AllGather", ALU.bypass, replica_groups=rg,
                        ins=[slabs[l + 1].opt()], outs=[Tfulls[l + 1].opt()],
                    )

            # ------------------------------------------------------------------
            # pooling allreduce + MLP
            nc.gpsimd.indirect_dma_start(
                out=poolb_in[:],
                out_offset=IndirectOffsetOnAxis(ap=poolidx_sb[:, :1], axis=0),
                in_=pool_sb[:],
                in_offset=None,
            )
            nc.gpsimd.collective_compute(
                "AllReduce", ALU.add, replica_groups=rg,
                ins=[poolb_in.opt()], outs=[poolb_out.opt()],
            )

            fW1_sb = [pp.tile([P, 768], F32, name=f"fW1_{kc}", tag=f"fW1_{kc}") for kc in range(7)]
            for kc in range(7):
                kr = min(P, PDA - kc * P)
                nc.sync.dma_start(fW1_sb[kc][:kr, :], fW1_d[kc * P:kc * P + kr, :])
            fb1_sb = pp.tile([P, 6], F32, tag="fb1")
            nc.sync.dma_start(fb1_sb[:], fb1_d[:])
            fW2_sb = pp.tile([P, 6], F32, tag="fW2")
            nc.sync.dma_start(fW2_sb[:], fW2_d[:])

            poolT = [pp.tile([P, max(G, P)], F32, name=f"poolT_{kc}", tag=f"poolT_{kc}") for kc in range(7)]
            for rt in range(0, G, P):
                rows = min(P, G - rt)
                prow = wpool.tile([P, PDA], F32, tag="prow")
                nc.sync.dma_start(prow[:rows, :], poolb_out[rt:rt + rows, :])
                for cb in range(7):
                    w = min(P, PDA - cb * P)
                    ptr2 = pmiscpool.tile([P, P], F32, tag="ptab")
                    nc.tensor.transpose(ptr2[:w, :rows], prow[:rows, cb * P:cb * P + w],
                                        identf[:rows, :rows])
                    nc.vector.tensor_copy(poolT[cb][:w, rt:rt + rows], ptr2[:w, :rows])

            h1_sb = [pp.tile([P, max(G, P)], F32, name=f"h1_{mo}", tag=f"h1_{mo}") for mo in range(6)]
            for mo in range(6):
                ph1 = pmiscpool.tile([P, max(G, P)], F32, tag="ptab")
                for kc in range(7):
                    kr = min(P, PDA - kc * P)
                    nc.tensor.matmul(ph1[:, :G], fW1_sb[kc][:kr, mo * P:(mo + 1) * P],
                                     poolT[kc][:kr, :G], start=(kc == 0), stop=(kc == 6))
                nc.scalar.activation(h1_sb[mo][:, :G], ph1[:, :G], AF.Relu,
                                     bias=fb1_sb[:, mo:mo + 1])

            py = pmiscpool.tile([1, max(G, P)], F32, tag="ppool")
            for mo in range(6):
                nc.tensor.matmul(py[:, :G], fW2_sb[:, mo:mo + 1], h1_sb[mo][:, :G],
                                 start=(mo == 0), stop=(mo == 5))
            ysb = wpool.tile([1, max(G, P)], F32, tag="ysb")
            nc.vector.tensor_copy(ysb[:, :G], py[:, :G])
            nc.sync.dma_start(y_d[:], ysb[:1, :G])

    nc.compile()
    return nc


# ----------------------------------------------------------------------------
# host preprocessing
# ----------------------------------------------------------------------------

def preprocess(inputs: dict, cfg: Cfg):
    n, g, ndev = cfg.n, cfg.g, cfg.ndev
    NPD, NT = cfg.npd, cfg.nt

    x = np.asarray(inputs["x"], np.float32)
    ei = np.asarray(inputs["edge_index"]).astype(np.int64)
    batch = np.asarray(inputs["batch"]).astype(np.int64)

    src = np.concatenate([ei[0], np.arange(n)])
    dst = np.concatenate([ei[1], np.arange(n)])
    order = np.argsort(dst, kind="stable")
    src, dst = src[order], dst[order]

    # per (dev, tile) edge lists
    tile_of = dst // P              # global dst tile id (NT per device)
    counts = np.bincount(tile_of, minlength=(n // P))
    nchunk = int(np.ceil(counts.max() / P))
    nchunk = ((nchunk + 3) // 4) * 4
    cfg.nchunk = nchunk
    NE = nchunk * P

    tile_start = np.zeros(n // P + 1, np.int64)
    np.cumsum(counts, out=tile_start[1:])

    def wrap_idx(a):  # [NE] int16 -> [128, NE // 16]
        w = a.reshape(-1, 16).T.copy()          # [16, NE // 16]
        return np.tile(w, (8, 1))               # [128, NE // 16]

    in_maps = []
    consts = {
        "identbf": np.eye(P, dtype=BF),
        "identf": np.eye(P, dtype=np.float32),
        "iotarow2": np.tile(np.arange(P, dtype=BF)[None, :], (P, 2)),
        "iotacol": np.arange(P, dtype=np.float32)[:, None],
        "ones1p": np.ones((1, P), BF),
        "onescol": np.ones((P, 1), BF),
        "onesrow1": np.ones((1, 1), BF),
    }
    att_all = np.stack([np.asarray(inputs[f"att{l+1}"], np.float32) for l in range(3)])  # [3, H, C]
    att_cols = np.zeros((C, 6), np.float32)
    for l in range(3):
        for h in range(H):
            att_cols[:, l * 2 + h] = att_all[l, h]

    b = [np.asarray(inputs[f"b{l+1}"], np.float32) for l in range(3)]
    wmats = {}
    for l in range(3):
        wmats[f"wl{l}"] = np.asarray(inputs[f"Wl{l+1}"], np.float32).astype(BF)
        wmats[f"wr{l}"] = np.asarray(inputs[f"Wr{l+1}"], np.float32).astype(BF)
    for l in (1, 2):
        wmats[f"rl{l}"] = (b[l - 1] @ np.asarray(inputs[f"Wl{l+1}"], np.float32))[None, :].astype(BF)
        wmats[f"rr{l}"] = (b[l - 1] @ np.asarray(inputs[f"Wr{l+1}"], np.float32))[None, :].astype(BF)

    b_all = np.concatenate(b)
    fW1 = np.asarray(inputs["fW1"], np.float32)
    fW1p = np.zeros((cfg.pda, 768), np.float32)
    fW1p[:768] = fW1
    fW1p[768] = b_all @ fW1
    fb1 = np.asarray(inputs["fb1"], np.float32).reshape(6, P).T.copy()
    fW2p = np.asarray(inputs["fW2"], np.float32).reshape(6, P).T.copy()
    fb2 = float(np.asarray(inputs["fb2"]).reshape(-1)[0])

    for dev in range(ndev):
        lo = dev * NPD
        g_lo = int(batch[lo])
        g_hi = int(batch[lo + NPD - 1])
        assert g_hi - g_lo + 1 <= P

        idxw = np.zeros((NT, P, nchunk * 8), np.int16)
        dstloc = np.full((NT, P, nchunk), -1.0, np.float32)
        dstrow = np.full((NT, NE), -1.0, np.float32)
        for t in range(NT):
            gt = dev * NT + t
            s, e = tile_start[gt], tile_start[gt + 1]
            cnt = e - s
            sp = np.zeros(NE, np.int64)
            sp[:cnt] = src[s:e]
            dl = np.full(NE, -1.0, np.float32)
            dl[:cnt] = (dst[s:e] % P).astype(np.float32)
            idxw[t] = wrap_idx(sp.astype(np.int16))
            dstloc[t] = dl.reshape(nchunk, P).T
            dstrow[t] = dl

        bloc = (batch[lo:lo + NPD].reshape(NT, P).T - g_lo).astype(np.float32)
        poolidx = np.arange(P, dtype=np.int32) + g_lo
        poolidx[poolidx > g_hi] = g
        m = {
            "xT": x[lo:lo + NPD].T.astype(BF),
            "att": att_cols.astype(BF),
            "idxw": idxw,
            "dstloc": dstloc,
            "dstrow": dstrow.astype(BF),
            "bloc": bloc,
            "poolidx": poolidx[:, None],
            "fW1p": fW1p, "fb1p": fb1, "fW2p": fW2p,
            **consts, **wmats,
        }
        in_maps.append(m)
    return in_maps, fb2


def kernel_impl(inputs, trace=False, trace_kwargs=None):
    cfg = Cfg(n=16384, g=256, ndev=8, nchunk=0)
    in_maps, fb2 = preprocess(inputs, cfg)
    nc = build_program(cfg, fb2)
    res = run_bass_kernel_spmd(nc, in_maps, core_ids=list(range(cfg.ndev)),
                               trace=trace, **(trace_kwargs or {}))
    y = np.asarray(res.results[0]["y"], np.float32).reshape(cfg.g, 1)
    return y + fb2, res


def kernel(**inputs) -> np.ndarray:
    y, _ = kernel_impl(inputs)
    return y

